# revision 18
# baseline (speedup 1.0000x reference)
"""Multi-head attention (b=4, n=2048, dim=512, heads=8, d_head=64) on 8 TRN2 NeuronCores.

Sharding: core = 2*b + head_group. Data parallel over batch (4), tensor
parallel over heads (2 groups of 4). Each core computes the QKV projection
for its 4 heads, full attention, and a partial output projection (its
heads' rows of W_out); the host sums the two partials per batch.

Device pipeline per core, engine-balanced so the ScalarE exp stream
(~8.7us per (i-block, head) unit) paces everything while PE/DVE hide
under it:
  - qk^T = wqkT.T @ xT -> [512(o), 2048(n)] (partition dim = head-major d)
  - v = xT.T @ wvT -> [2048, 256], stored as [128, 4*65] tiles with a 1.0
    column per head so the PV matmul also produces the softmax denominator
  - per slot (unit, j-pair): S^T[j,i] = k^T.T @ q^T into PSUM; P~ =
    exp(S^T * scale) on ScalarE; PV *flipped*: U[i,65] += P~[j,i].T @
    [v_h|1][j,65] -- output partitions are i, so the denominator is a
    per-partition scalar (cheap batched reciprocal + tensor_scalar).
  - scores for slot k+2 are emitted at slot k (ps pool bufs=3), so ScalarE
    holds a two-slot lead and never starves while the PE chews fillers.
  - per n-tile: PE-transpose A -> AT [hd, n]; out = AT.T @ woT -> partial
    [2048, 512] f32 -> DMA out. Projections, transposes and output DMAs
    drip into the exp-paced gaps on an explicit (unit, j-pair) schedule.
"""

import functools
import sys

if "/opt/trn_rl_repo" not in sys.path:
    sys.path.insert(0, "/opt/trn_rl_repo")

import numpy as np
import ml_dtypes

import concourse.bacc as bacc
import concourse.mybir as mybir
import concourse.tile as tile
from concourse.bass_utils import run_bass_kernel_spmd

N_CORES = 8
B = 4
N = 2048          # sequence length
C = 512           # model dim
HPC = 4           # heads per core
D = 64            # head dim
SCALE = D ** -0.5

F32 = mybir.dt.float32
BF16 = mybir.dt.bfloat16

NT = N // 128     # 16 n/j tiles of 128
KT = C // 128     # 4 contraction tiles for the projections
IB = 4            # i-blocks of 512
JP = NT // 2      # 8 j-tile pairs per i-block


def _build_body(nc, tc, ctx, xT_d, wqkT_d, wvT_d, woT_d, ident_d, out_d):
    sb = ctx.enter_context(tc.tile_pool(name="sb", bufs=1))
    work = ctx.enter_context(tc.tile_pool(name="work", bufs=2))
    ptp = ctx.enter_context(tc.tile_pool(name="ptp", bufs=4))
    psp = ctx.enter_context(tc.tile_pool(name="psp", bufs=3, space="PSUM"))
    pup = ctx.enter_context(tc.tile_pool(name="pup", bufs=1, space="PSUM"))
    ppp = ctx.enter_context(tc.tile_pool(name="ppp", bufs=1, space="PSUM"))

    # ---- persistent SBUF tensors ----
    xT = [sb.tile([128, N], BF16, tag=f"x{k}", name=f"x{k}") for k in range(KT)]
    wqk = [sb.tile([128, 512], BF16, tag=f"wqk{k}", name=f"wqk{k}") for k in range(KT)]
    wv = [sb.tile([128, 256], BF16, tag=f"wv{k}", name=f"wv{k}") for k in range(KT)]
    wo = [sb.tile([128, 512], BF16, tag=f"wo{t}", name=f"wo{t}") for t in range(2)]
    ident = sb.tile([128, 128], BF16, tag="ident", name="ident")
    qkT = [sb.tile([128, N], BF16, tag=f"qk{o}", name=f"qk{o}") for o in range(4)]
    vsb = [sb.tile([128, HPC * 65], BF16, tag=f"v{t}", name=f"v{t}") for t in range(NT)]
    Asb = [sb.tile([128, 256], BF16, tag=f"a{t}", name=f"a{t}") for t in range(NT)]
    ATsb = [sb.tile([128, N], BF16, tag=f"at{t}", name=f"at{t}") for t in range(2)]

    # warm the ACT exp table set at t=0 so the one-time ~1.3us table DMA
    # overlaps the input DMAs instead of delaying the first real exp
    warm = work.tile([128, 16], F32, tag="warm", name="warm", bufs=1)
    nc.vector.memset(warm[:], 0.0)
    nc.scalar.activation(warm[:], warm[:], mybir.ActivationFunctionType.Exp)

    # ---- input DMAs, ordered by first use: the wqk column slices feeding
    # the first two qk chunks land first, then the first 512-column slice
    # of each xT tile, then everything else ----
    for k in range(KT):
        nc.sync.dma_start(out=wqk[k][:, 256:384], in_=wqkT_d[k * 128:(k + 1) * 128, 256:384])
        nc.scalar.dma_start(out=wqk[k][:, 0:128], in_=wqkT_d[k * 128:(k + 1) * 128, 0:128])
    for k in range(KT):
        weng = nc.sync if k % 2 == 0 else nc.scalar
        weng.dma_start(out=xT[k][:, 0:512], in_=xT_d[k * 128:(k + 1) * 128, 0:512])
    for k in range(KT):
        nc.sync.dma_start(out=wqk[k][:, 128:256], in_=wqkT_d[k * 128:(k + 1) * 128, 128:256])
        nc.scalar.dma_start(out=wqk[k][:, 384:512], in_=wqkT_d[k * 128:(k + 1) * 128, 384:512])
    for k in range(KT):
        nc.sync.dma_start(out=wv[k][:], in_=wvT_d[k * 128:(k + 1) * 128, :])
    for nch in range(1, 4):
        for k in range(KT):
            nc.sync.dma_start(
                out=xT[k][:, nch * 512:(nch + 1) * 512],
                in_=xT_d[k * 128:(k + 1) * 128, nch * 512:(nch + 1) * 512],
            )
    for t in range(2):
        nc.sync.dma_start(out=wo[t][:], in_=woT_d[t * 128:(t + 1) * 128, :])
    nc.scalar.dma_start(out=ident[:], in_=ident_d[:, :])

    # ones columns of v tiles (never overwritten by the v eviction)
    for t in range(NT):
        v3 = vsb[t][:].rearrange("p (h c) -> p h c", c=65)
        nc.vector.memset(v3[:, :, 64:65], 1.0)

    # ---- filler emitters (dripped into the exp-paced gaps) ----
    # qk o-tiles: 0 = q heads 0/1, 1 = q heads 2/3, 2 = k heads 0/1, 3 = k 2/3.
    def qk_chunk(ot, nch):
        def f():
            pp = ppp.tile([128, 512], F32, tag="pp", name="pp")
            for k in range(KT):
                nc.tensor.matmul(
                    pp[:, 0:512],
                    wqk[k][:, ot * 128:(ot + 1) * 128],
                    xT[k][:, nch * 512:(nch + 1) * 512],
                    start=(k == 0),
                    stop=(k == KT - 1),
                )
            nc.vector.tensor_copy(qkT[ot][:, nch * 512:(nch + 1) * 512], pp[:, 0:512])
        return f

    def v_pair(tp, hp):
        # v projection for head-pair hp (columns hp*128:(hp+1)*128 of wv);
        # both tiles share one PSUM buffer in disjoint regions so the
        # second tile's matmuls don't wait on the first tile's cast
        def f():
            pp = ppp.tile([128, 512], F32, tag="pp", name="ppv")
            for i, t in enumerate((2 * tp, 2 * tp + 1)):
                for k in range(KT):
                    nc.tensor.matmul(
                        pp[:, i * 128:(i + 1) * 128],
                        xT[k][:, t * 128:(t + 1) * 128],
                        wv[k][:, hp * 128:(hp + 1) * 128],
                        start=(k == 0),
                        stop=(k == KT - 1),
                    )
            for i, t in enumerate((2 * tp, 2 * tp + 1)):
                v3 = vsb[t][:].rearrange("p (h c) -> p h c", c=65)
                p3 = pp[:, i * 128:(i + 1) * 128].rearrange("p (h c) -> p h c", c=64)
                nc.vector.tensor_copy(v3[:, 2 * hp:2 * hp + 2, 0:64], p3)
        return f

    def out_transpose(nt):
        def f():
            tp = ppp.tile([128, 512], BF16, tag="pp", name="tp")
            for t2 in range(2):
                nc.tensor.transpose(
                    tp[:, t2 * 128:(t2 + 1) * 128],
                    Asb[nt][:, t2 * 128:(t2 + 1) * 128], ident[:])
            for t2 in range(2):
                nc.vector.tensor_copy(
                    ATsb[t2][:, nt * 128:(nt + 1) * 128],
                    tp[:, t2 * 128:(t2 + 1) * 128])
        return f

    def out_piece(nt, h):
        # per-head transpose piece: [128 i, 64] -> AT rows (h%2)*64
        def f():
            tp = ppp.tile([128, 512], BF16, tag="pp", name="tpp")
            nc.tensor.transpose(
                tp[0:64, 0:128], Asb[nt][:, h * 64:(h + 1) * 64], ident[:])
            nc.vector.tensor_copy(
                ATsb[h // 2][(h % 2) * 64:(h % 2) * 64 + 64,
                             nt * 128:(nt + 1) * 128],
                tp[0:64, 0:128],
            )
        return f

    def out_proj(nt):
        def f():
            ppo = ppp.tile([128, 512], F32, tag="pp", name="ppo")
            for t2 in range(2):
                nc.tensor.matmul(
                    ppo[:, 0:512],
                    ATsb[t2][:, nt * 128:(nt + 1) * 128],
                    wo[t2][:],
                    start=(t2 == 0),
                    stop=(t2 == 1),
                )
            osb = work.tile([128, 512], F32, tag="osb", name="osb")
            nc.vector.tensor_copy(osb[:], ppo[:, 0:512])
            eng = (nc.sync, nc.gpsimd)[nt % 2]
            eng.dma_start(out=out_d[nt * 128:(nt + 1) * 128, :], in_=osb[:])
        return f

    # ---- unit order: head-pair 0/1 over all i-blocks first, then 2/3, so
    # the q/k projections for heads 2/3 drip in long after the fill phase ----
    units = [(ib, hp * 2 + h) for hp in range(2) for ib in range(IB)
             for h in range(2)]

    # ---- filler schedule: (unit, jp) -> closures, emitted right after the
    # exp so the PE chews them while ScalarE works and PV waits its sem ----
    sched = {}

    def at(u, jp, f):
        sched.setdefault((u, jp), []).append(f)

    # NOTE: S(k) is emitted two slots early (at slot k-2), so a qk chunk
    # feeding S(u, jp) must be scheduled strictly before slot (u, jp-2).
    for jp in range(JP):
        at(0, jp, v_pair(jp, 0))              # h0/h1 V, consumed by u0's PV
    at(0, 1, qk_chunk(2, 2))                  # k h0/1 chunk 2, S emitted at (0,2)
    at(0, 3, qk_chunk(2, 3))                  # k h0/1 chunk 3, S emitted at (0,4)
    at(1, 0, qk_chunk(0, 1))                  # q h0/1 chunk 1, due u2
    at(3, 0, qk_chunk(0, 2))                  # due u4
    at(5, 0, qk_chunk(0, 3))                  # due u6
    for tp in range(4):
        at(4, 1 + tp, v_pair(tp, 1))          # h2/h3 V, due u8
        at(5, 1 + tp, v_pair(4 + tp, 1))
    at(5, 5, qk_chunk(3, 0))                  # k h2/3, due u8
    at(6, 2, qk_chunk(3, 1))
    at(6, 5, qk_chunk(3, 2))                  # due u8.jp4
    at(7, 5, qk_chunk(3, 3))                  # due u8.jp6
    at(7, 4, qk_chunk(1, 0))                  # q h2/3 chunk 0, S(u8,0) at (7,6)
    at(9, 0, qk_chunk(1, 1))                  # due u10
    at(11, 0, qk_chunk(1, 2))                 # due u12
    at(13, 0, qk_chunk(1, 3))                 # due u14
    # group-3 A slices transpose per head as soon as each head finishes
    # (units 6, 7, 14 for h0, h1, h2); h3 lands in the tail
    for c in range(4):
        at(7, c, out_piece(12 + c, 0))
        at(8, c, out_piece(12 + c, 1))
        at(15, c, out_piece(12 + c, 2))
    # groups 0-2 complete at units 9, 11, 13; whole-tile transposes + proj
    for g in range(3):
        for c in range(4):
            at(10 + 2 * g, c, out_transpose(4 * g + c))
        for c in range(3):
            at(10 + 2 * g, 4 + c, out_proj(4 * g + c))
        at(11 + 2 * g, 4, out_proj(4 * g + 3))

    # ---- main pipeline, software-pipelined two slots deep ----
    exp_t = mybir.ActivationFunctionType.Exp

    slots = [(ui, jp) for ui in range(len(units)) for jp in range(JP)]
    ps_tiles = {}

    def emit_S(k):
        ui, jp = slots[k]
        ib, h = units[ui]
        rows = slice((h % 2) * 64, (h % 2) * 64 + 64)
        ps = psp.tile([128, 1024], F32, tag="ps", name="ps")
        for half in range(2):
            jt = jp * 2 + half
            nc.tensor.matmul(
                ps[:, half * 512:(half + 1) * 512],
                qkT[2 + h // 2][rows, jt * 128:(jt + 1) * 128],
                qkT[h // 2][rows, ib * 512:(ib + 1) * 512],
                start=True,
                stop=True,
            )
        ps_tiles[k] = ps

    qk_chunk(2, 0)()
    qk_chunk(0, 0)()
    emit_S(0)
    qk_chunk(2, 1)()                          # k h0/1 chunk 1: S(0,2) is
    emit_S(1)                                 # emitted at slot (0,0)

    pu = None
    for k, (ui, jp) in enumerate(slots):
        ib, h = units[ui]
        ps = ps_tiles.pop(k)
        pt = ptp.tile([128, 1024], BF16, tag="pt", name="pt")
        nc.scalar.activation(pt[:], ps[:], exp_t, scale=SCALE)
        if k + 2 < len(slots):
            emit_S(k + 2)
        for f in sched.get((ui, jp), ()):
            f()
        if jp == 0:
            pu = pup.tile([128, 260], F32, tag="pu", name="pu")
        for half in range(2):
            jt = jp * 2 + half
            for c in range(4):
                # start=True clears the whole PSUM bank, so only the very
                # first matmul of the unit sets it; the other i-chunks'
                # first writes rely on per-element has_written overwrite.
                nc.tensor.matmul(
                    pu[:, c * 65:(c + 1) * 65],
                    pt[:, half * 512 + c * 128:half * 512 + (c + 1) * 128],
                    vsb[jt][:, h * 65:(h + 1) * 65],
                    start=(jt == 0 and c == 0),
                    stop=(jt == NT - 1),
                    skip_group_check=True,
                )
        if jp == JP - 1:
            # evacuate PSUM fast (pu bufs=1: the next unit's PV rotation
            # only waits on this one copy), normalize from the SBUF copy
            usb = work.tile([128, 260], F32, tag="usb", name="usb")
            nc.vector.tensor_copy(usb[:], pu[:])
            for c in range(4):
                rec = work.tile([128, 1], F32, tag="rec", name="rec")
                nc.vector.reciprocal(rec[:], usb[:, c * 65 + 64:c * 65 + 65])
                nc.vector.tensor_scalar_mul(
                    Asb[ib * 4 + c][:, h * 64:(h + 1) * 64],
                    usb[:, c * 65:c * 65 + 64],
                    rec[:],
                )
    for nt in range(12, 16):
        out_piece(nt, 3)()
    for nt in range(12, 16):
        out_proj(nt)()


@functools.lru_cache(maxsize=1)
def _build():
    nc = bacc.Bacc("TRN2", target_bir_lowering=False, debug=False,
                   num_devices=N_CORES)
    xT_d = nc.dram_tensor("xT", [C, N], BF16, kind="ExternalInput").ap()
    wqkT_d = nc.dram_tensor("wqkT", [C, 512], BF16, kind="ExternalInput").ap()
    wvT_d = nc.dram_tensor("wvT", [C, 256], BF16, kind="ExternalInput").ap()
    woT_d = nc.dram_tensor("woT", [256, C], BF16, kind="ExternalInput").ap()
    ident_d = nc.dram_tensor("ident", [128, 128], BF16, kind="ExternalInput").ap()
    out_d = nc.dram_tensor("out", [N, C], F32, kind="ExternalOutput").ap()
    from contextlib import ExitStack
    with tile.TileContext(nc) as tc, ExitStack() as ctx:
        _build_body(nc, tc, ctx, xT_d, wqkT_d, wvT_d, woT_d, ident_d, out_d)
    nc.compile()
    return nc


def _shard_inputs(x, W_qkv, W_out):
    bf16 = ml_dtypes.bfloat16
    ident = np.eye(128, dtype=bf16)
    in_maps = []
    for core in range(N_CORES):
        b, hg = core // 2, core % 2
        xT = np.ascontiguousarray(x[b].T).astype(bf16)
        rows_q = W_qkv[hg * 256:(hg + 1) * 256, :]
        rows_k = W_qkv[512 + hg * 256:512 + (hg + 1) * 256, :]
        wqkT = np.ascontiguousarray(
            np.concatenate([rows_q, rows_k], 0).T).astype(bf16)
        wvT = np.ascontiguousarray(
            W_qkv[1024 + hg * 256:1024 + (hg + 1) * 256, :].T).astype(bf16)
        woT = np.ascontiguousarray(
            W_out[:, hg * 256:(hg + 1) * 256].T).astype(bf16)
        in_maps.append(
            {"xT": xT, "wqkT": wqkT, "wvT": wvT, "woT": woT, "ident": ident})
    return in_maps


def _run(inputs, trace=False, tmpdir=None):
    x = np.asarray(inputs["x"], dtype=np.float32)
    W_qkv = np.asarray(inputs["W_qkv"], dtype=np.float32)
    W_out = np.asarray(inputs["W_out"], dtype=np.float32)
    nc = _build()
    in_maps = _shard_inputs(x, W_qkv, W_out)
    kwargs = {}
    if trace:
        kwargs = dict(trace=True, tmpdir=tmpdir)
    res = run_bass_kernel_spmd(nc, in_maps, core_ids=list(range(N_CORES)), **kwargs)
    out = np.zeros((B, N, C), np.float32)
    for core in range(N_CORES):
        out[core // 2] += res.results[core]["out"]
    return out, res


def kernel(**inputs):
    out, _ = _run(inputs)
    return out


# revision 22
# speedup vs baseline: 1.0518x; 1.0518x over previous
"""Multi-head attention (b=4, n=2048, dim=512, heads=8, d_head=64) on 8 TRN2 NeuronCores.

Sharding: core = 2*b + head_group. Data parallel over batch (4), tensor
parallel over heads (2 groups of 4). Each core computes the QKV projection
for its 4 heads, full attention, and a partial output projection (its
heads' rows of W_out); the host sums the two partials per batch.

Device pipeline per core, engine-balanced so the ScalarE exp stream
(~8.7us per (i-block, head) unit) paces everything while PE/DVE hide
under it:
  - qk^T = wqkT.T @ xT -> [512(o), 2048(n)] (partition dim = head-major d)
  - v = xT.T @ wvT -> [2048, 256], stored as [128, 4*65] tiles with a 1.0
    column per head so the PV matmul also produces the softmax denominator
  - per slot (unit, j-pair): S^T[j,i] = k^T.T @ q^T into PSUM; P~ =
    exp(S^T * scale) on ScalarE; PV *flipped*: U[i,65] += P~[j,i].T @
    [v_h|1][j,65] -- output partitions are i, so the denominator is a
    per-partition scalar (cheap batched reciprocal + tensor_scalar).
  - scores for slot k+2 are emitted at slot k (ps pool bufs=3), so ScalarE
    holds a two-slot lead and never starves while the PE chews fillers.
  - per n-tile: PE-transpose A -> AT [hd, n]; out = AT.T @ woT -> partial
    [2048, 512] f32 -> DMA out. Projections, transposes and output DMAs
    drip into the exp-paced gaps on an explicit (unit, j-pair) schedule.
"""

import functools
import sys

if "/opt/trn_rl_repo" not in sys.path:
    sys.path.insert(0, "/opt/trn_rl_repo")

import numpy as np
import ml_dtypes

import concourse.bacc as bacc
import concourse.mybir as mybir
import concourse.tile as tile
from concourse.bass_utils import run_bass_kernel_spmd

N_CORES = 8
B = 4
N = 2048          # sequence length
C = 512           # model dim
HPC = 4           # heads per core
D = 64            # head dim
SCALE = D ** -0.5

F32 = mybir.dt.float32
BF16 = mybir.dt.bfloat16

NT = N // 128     # 16 n/j tiles of 128
KT = C // 128     # 4 contraction tiles for the projections
IB = 4            # i-blocks of 512
JP = NT // 2      # 8 j-tile pairs per i-block


def _build_body(nc, tc, ctx, xT_d, wqkT_d, wvT_d, woT_d, ident_d, out_d):
    sb = ctx.enter_context(tc.tile_pool(name="sb", bufs=1))
    work = ctx.enter_context(tc.tile_pool(name="work", bufs=2))
    ptp = ctx.enter_context(tc.tile_pool(name="ptp", bufs=4))
    psp = ctx.enter_context(tc.tile_pool(name="psp", bufs=2, space="PSUM"))
    pup = ctx.enter_context(tc.tile_pool(name="pup", bufs=2, space="PSUM"))
    ppp = ctx.enter_context(tc.tile_pool(name="ppp", bufs=2, space="PSUM"))

    # ---- persistent SBUF tensors ----
    xT = [sb.tile([128, N], BF16, tag=f"x{k}", name=f"x{k}") for k in range(KT)]
    wqk = [sb.tile([128, 512], BF16, tag=f"wqk{k}", name=f"wqk{k}") for k in range(KT)]
    wv = [sb.tile([128, 256], BF16, tag=f"wv{k}", name=f"wv{k}") for k in range(KT)]
    wo = [sb.tile([128, 512], BF16, tag=f"wo{t}", name=f"wo{t}") for t in range(2)]
    ident = sb.tile([128, 128], BF16, tag="ident", name="ident")
    qkT = [sb.tile([128, N], BF16, tag=f"qk{o}", name=f"qk{o}") for o in range(4)]
    vsb = [sb.tile([128, HPC * 65], BF16, tag=f"v{t}", name=f"v{t}") for t in range(NT)]
    Asb = [sb.tile([128, 256], BF16, tag=f"a{t}", name=f"a{t}") for t in range(NT)]
    ATsb = [sb.tile([128, N], BF16, tag=f"at{t}", name=f"at{t}") for t in range(2)]

    # warm the ACT exp table set at t=0 so the one-time ~1.3us table DMA
    # overlaps the input DMAs instead of delaying the first real exp
    warm = work.tile([128, 16], F32, tag="warm", name="warm", bufs=1)
    nc.vector.memset(warm[:], 0.0)
    nc.scalar.activation(warm[:], warm[:], mybir.ActivationFunctionType.Exp)

    # ---- input DMAs on three queues (sync, scalar, gpsimd), ordered by
    # first use: wqk + xT chunk 0 feed the prologue qk chunks, wv rides the
    # gpsimd queue in parallel so unit 0's v fillers never stall the PE ----
    for k in range(KT):
        weng = nc.sync if k % 2 == 0 else nc.scalar
        weng.dma_start(out=wqk[k][:], in_=wqkT_d[k * 128:(k + 1) * 128, :])
        weng.dma_start(out=xT[k][:, 0:512], in_=xT_d[k * 128:(k + 1) * 128, 0:512])
    for k in range(KT):
        nc.gpsimd.dma_start(out=wv[k][:], in_=wvT_d[k * 128:(k + 1) * 128, :])
    for nch in range(1, 4):
        for k in range(KT):
            eng = (nc.sync, nc.scalar, nc.gpsimd)[(nch * KT + k) % 3]
            eng.dma_start(
                out=xT[k][:, nch * 512:(nch + 1) * 512],
                in_=xT_d[k * 128:(k + 1) * 128, nch * 512:(nch + 1) * 512],
            )
    for t in range(2):
        nc.gpsimd.dma_start(out=wo[t][:], in_=woT_d[t * 128:(t + 1) * 128, :])
    nc.gpsimd.dma_start(out=ident[:], in_=ident_d[:, :])

    # ones columns of v tiles (never overwritten by the v eviction)
    for t in range(NT):
        v3 = vsb[t][:].rearrange("p (h c) -> p h c", c=65)
        nc.vector.memset(v3[:, :, 64:65], 1.0)

    # ---- filler emitters (dripped into the exp-paced gaps) ----
    # qk o-tiles: 0 = q heads 0/1, 1 = q heads 2/3, 2 = k heads 0/1, 3 = k 2/3.
    def qk_chunk(ot, nch):
        def f():
            pp = ppp.tile([128, 512], F32, tag="pp", name="pp")
            for k in range(KT):
                nc.tensor.matmul(
                    pp[:, 0:512],
                    wqk[k][:, ot * 128:(ot + 1) * 128],
                    xT[k][:, nch * 512:(nch + 1) * 512],
                    start=(k == 0),
                    stop=(k == KT - 1),
                )
            nc.vector.tensor_copy(qkT[ot][:, nch * 512:(nch + 1) * 512], pp[:, 0:512])
        return f

    def v_pair(tp, hp):
        # v projection for head-pair hp (columns hp*128:(hp+1)*128 of wv);
        # both tiles share one PSUM buffer in disjoint regions so the
        # second tile's matmuls don't wait on the first tile's cast
        def f():
            pp = ppp.tile([128, 512], F32, tag="pp", name="ppv")
            for i, t in enumerate((2 * tp, 2 * tp + 1)):
                for k in range(KT):
                    nc.tensor.matmul(
                        pp[:, i * 128:(i + 1) * 128],
                        xT[k][:, t * 128:(t + 1) * 128],
                        wv[k][:, hp * 128:(hp + 1) * 128],
                        start=(k == 0),
                        stop=(k == KT - 1),
                    )
            for i, t in enumerate((2 * tp, 2 * tp + 1)):
                v3 = vsb[t][:].rearrange("p (h c) -> p h c", c=65)
                p3 = pp[:, i * 128:(i + 1) * 128].rearrange("p (h c) -> p h c", c=64)
                nc.vector.tensor_copy(v3[:, 2 * hp:2 * hp + 2, 0:64], p3)
        return f

    def out_transpose(nt):
        def f():
            tp = ppp.tile([128, 512], BF16, tag="pp", name="tp")
            for t2 in range(2):
                nc.tensor.transpose(
                    tp[:, t2 * 128:(t2 + 1) * 128],
                    Asb[nt][:, t2 * 128:(t2 + 1) * 128], ident[:])
            for t2 in range(2):
                nc.vector.tensor_copy(
                    ATsb[t2][:, nt * 128:(nt + 1) * 128],
                    tp[:, t2 * 128:(t2 + 1) * 128])
        return f

    def out_piece(nt, h):
        # per-head transpose piece: [128 i, 64] -> AT rows (h%2)*64
        def f():
            tp = ppp.tile([128, 512], BF16, tag="pp", name="tpp")
            nc.tensor.transpose(
                tp[0:64, 0:128], Asb[nt][:, h * 64:(h + 1) * 64], ident[:])
            nc.vector.tensor_copy(
                ATsb[h // 2][(h % 2) * 64:(h % 2) * 64 + 64,
                             nt * 128:(nt + 1) * 128],
                tp[0:64, 0:128],
            )
        return f

    def out_proj(nt):
        def f():
            ppo = ppp.tile([128, 512], F32, tag="pp", name="ppo")
            for t2 in range(2):
                nc.tensor.matmul(
                    ppo[:, 0:512],
                    ATsb[t2][:, nt * 128:(nt + 1) * 128],
                    wo[t2][:],
                    start=(t2 == 0),
                    stop=(t2 == 1),
                )
            osb = work.tile([128, 512], F32, tag="osb", name="osb")
            nc.vector.tensor_copy(osb[:], ppo[:, 0:512])
            eng = (nc.sync, nc.gpsimd)[nt % 2]
            eng.dma_start(out=out_d[nt * 128:(nt + 1) * 128, :], in_=osb[:])
        return f

    # ---- unit order: head-pair 0/1 over all i-blocks first, then 2/3, so
    # the q/k projections for heads 2/3 drip in long after the fill phase ----
    units = [(ib, hp * 2 + h) for hp in range(2) for ib in range(IB)
             for h in range(2)]

    # ---- filler schedule: (unit, jp) -> closures, emitted right after the
    # exp so the PE chews them while ScalarE works and PV waits its sem ----
    sched = {}

    def at(u, jp, f):
        sched.setdefault((u, jp), []).append(f)

    # NOTE: S(k) is emitted two slots early (at slot k-2), so a qk chunk
    # feeding S(u, jp) must be scheduled strictly before slot (u, jp-2).
    for jp in range(JP):
        at(0, jp, v_pair(jp, 0))              # h0/h1 V, consumed by u0's PV
    at(0, 1, qk_chunk(2, 2))                  # k h0/1 chunk 2, S emitted at (0,2)
    at(0, 3, qk_chunk(2, 3))                  # k h0/1 chunk 3, S emitted at (0,4)
    at(1, 0, qk_chunk(0, 1))                  # q h0/1 chunk 1, due u2
    at(3, 0, qk_chunk(0, 2))                  # due u4
    at(5, 0, qk_chunk(0, 3))                  # due u6
    for tp in range(4):
        at(4, 1 + tp, v_pair(tp, 1))          # h2/h3 V, due u8
        at(5, 1 + tp, v_pair(4 + tp, 1))
    at(5, 5, qk_chunk(3, 0))                  # k h2/3, due u8
    at(6, 2, qk_chunk(3, 1))
    at(6, 5, qk_chunk(3, 2))                  # due u8.jp4
    at(7, 5, qk_chunk(3, 3))                  # due u8.jp6
    at(7, 4, qk_chunk(1, 0))                  # q h2/3 chunk 0, S(u8,0) at (7,6)
    at(9, 0, qk_chunk(1, 1))                  # due u10
    at(11, 0, qk_chunk(1, 2))                 # due u12
    at(13, 0, qk_chunk(1, 3))                 # due u14
    # group-3 A slices transpose per head as soon as each head finishes
    # (units 6, 7, 14 for h0, h1, h2); h3 lands in the tail
    for c in range(4):
        at(7, c, out_piece(12 + c, 0))
        at(8, c, out_piece(12 + c, 1))
        at(15, c, out_piece(12 + c, 2))
    # groups 0-2 complete at units 9, 11, 13; whole-tile transposes + proj
    for g in range(3):
        for c in range(4):
            at(10 + 2 * g, c, out_transpose(4 * g + c))
        for c in range(3):
            at(10 + 2 * g, 4 + c, out_proj(4 * g + c))
        at(11 + 2 * g, 4, out_proj(4 * g + 3))

    # ---- main pipeline, software-pipelined two slots deep ----
    exp_t = mybir.ActivationFunctionType.Exp

    slots = [(ui, jp) for ui in range(len(units)) for jp in range(JP)]
    ps_tiles = {}

    def emit_S(k):
        ui, jp = slots[k]
        ib, h = units[ui]
        rows = slice((h % 2) * 64, (h % 2) * 64 + 64)
        ps = psp.tile([128, 1024], F32, tag="ps", name="ps")
        for half in range(2):
            jt = jp * 2 + half
            nc.tensor.matmul(
                ps[:, half * 512:(half + 1) * 512],
                qkT[2 + h // 2][rows, jt * 128:(jt + 1) * 128],
                qkT[h // 2][rows, ib * 512:(ib + 1) * 512],
                start=True,
                stop=True,
            )
        ps_tiles[k] = ps

    qk_chunk(2, 0)()
    qk_chunk(0, 0)()
    emit_S(0)
    emit_S(1)
    qk_chunk(2, 1)()                          # k h0/1 chunk 1, before S(0,2)
                                              # which is emitted at slot (0,0)

    pu = None
    for k, (ui, jp) in enumerate(slots):
        ib, h = units[ui]
        ps = ps_tiles.pop(k)
        pt = ptp.tile([128, 1024], BF16, tag="pt", name="pt")
        nc.scalar.activation(pt[:], ps[:], exp_t, scale=SCALE)
        if k + 2 < len(slots):
            emit_S(k + 2)
        for f in sched.get((ui, jp), ()):
            f()
        if jp == 0:
            pu = pup.tile([128, 260], F32, tag="pu", name="pu")
        for half in range(2):
            jt = jp * 2 + half
            for c in range(4):
                # start=True clears the whole PSUM bank, so only the very
                # first matmul of the unit sets it; the other i-chunks'
                # first writes rely on per-element has_written overwrite.
                nc.tensor.matmul(
                    pu[:, c * 65:(c + 1) * 65],
                    pt[:, half * 512 + c * 128:half * 512 + (c + 1) * 128],
                    vsb[jt][:, h * 65:(h + 1) * 65],
                    start=(jt == 0 and c == 0),
                    stop=(jt == NT - 1),
                    skip_group_check=True,
                )
        if jp == JP - 1:
            for c in range(4):
                rec = work.tile([128, 1], F32, tag="rec", name="rec")
                nc.vector.reciprocal(rec[:], pu[:, c * 65 + 64:c * 65 + 65])
                nc.vector.tensor_scalar_mul(
                    Asb[ib * 4 + c][:, h * 64:(h + 1) * 64],
                    pu[:, c * 65:c * 65 + 64],
                    rec[:],
                )
    for nt in range(12, 16):
        out_piece(nt, 3)()
    for nt in range(12, 16):
        out_proj(nt)()


@functools.lru_cache(maxsize=1)
def _build():
    nc = bacc.Bacc("TRN2", target_bir_lowering=False, debug=False,
                   num_devices=N_CORES)
    xT_d = nc.dram_tensor("xT", [C, N], BF16, kind="ExternalInput").ap()
    wqkT_d = nc.dram_tensor("wqkT", [C, 512], BF16, kind="ExternalInput").ap()
    wvT_d = nc.dram_tensor("wvT", [C, 256], BF16, kind="ExternalInput").ap()
    woT_d = nc.dram_tensor("woT", [256, C], BF16, kind="ExternalInput").ap()
    ident_d = nc.dram_tensor("ident", [128, 128], BF16, kind="ExternalInput").ap()
    out_d = nc.dram_tensor("out", [N, C], F32, kind="ExternalOutput").ap()
    from contextlib import ExitStack
    with tile.TileContext(nc) as tc, ExitStack() as ctx:
        _build_body(nc, tc, ctx, xT_d, wqkT_d, wvT_d, woT_d, ident_d, out_d)
    nc.compile()
    return nc


def _shard_inputs(x, W_qkv, W_out):
    bf16 = ml_dtypes.bfloat16
    ident = np.eye(128, dtype=bf16)
    in_maps = []
    for core in range(N_CORES):
        b, hg = core // 2, core % 2
        xT = np.ascontiguousarray(x[b].T).astype(bf16)
        rows_q = W_qkv[hg * 256:(hg + 1) * 256, :]
        rows_k = W_qkv[512 + hg * 256:512 + (hg + 1) * 256, :]
        wqkT = np.ascontiguousarray(
            np.concatenate([rows_q, rows_k], 0).T).astype(bf16)
        wvT = np.ascontiguousarray(
            W_qkv[1024 + hg * 256:1024 + (hg + 1) * 256, :].T).astype(bf16)
        woT = np.ascontiguousarray(
            W_out[:, hg * 256:(hg + 1) * 256].T).astype(bf16)
        in_maps.append(
            {"xT": xT, "wqkT": wqkT, "wvT": wvT, "woT": woT, "ident": ident})
    return in_maps


def _run(inputs, trace=False, tmpdir=None):
    x = np.asarray(inputs["x"], dtype=np.float32)
    W_qkv = np.asarray(inputs["W_qkv"], dtype=np.float32)
    W_out = np.asarray(inputs["W_out"], dtype=np.float32)
    nc = _build()
    in_maps = _shard_inputs(x, W_qkv, W_out)
    kwargs = {}
    if trace:
        kwargs = dict(trace=True, tmpdir=tmpdir)
    res = run_bass_kernel_spmd(nc, in_maps, core_ids=list(range(N_CORES)), **kwargs)
    out = np.zeros((B, N, C), np.float32)
    for core in range(N_CORES):
        out[core // 2] += res.results[core]["out"]
    return out, res


def kernel(**inputs):
    out, _ = _run(inputs)
    return out


# revision 24
# speedup vs baseline: 1.0780x; 1.0250x over previous
"""Multi-head attention (b=4, n=2048, dim=512, heads=8, d_head=64) on 8 TRN2 NeuronCores.

Sharding: core = 2*b + head_group. Data parallel over batch (4), tensor
parallel over heads (2 groups of 4). Each core computes the QKV projection
for its 4 heads, full attention, and a partial output projection (its
heads' rows of W_out); the host sums the two partials per batch.

Device pipeline per core, engine-balanced so the ScalarE exp stream
(~8.7us per (i-block, head) unit) paces everything while PE/DVE hide
under it:
  - qk^T = wqkT.T @ xT -> [512(o), 2048(n)] (partition dim = head-major d)
  - v = xT.T @ wvT -> [2048, 256], stored as [128, 4*65] tiles with a 1.0
    column per head so the PV matmul also produces the softmax denominator
  - per slot (unit, j-pair): S^T[j,i] = k^T.T @ q^T into PSUM; P~ =
    exp(S^T * scale) on ScalarE; PV *flipped*: U[i,65] += P~[j,i].T @
    [v_h|1][j,65] -- output partitions are i, so the denominator is a
    per-partition scalar (cheap batched reciprocal + tensor_scalar).
  - scores for slot k+2 are emitted at slot k (ps pool bufs=3), so ScalarE
    holds a two-slot lead and never starves while the PE chews fillers.
  - per n-tile: PE-transpose A -> AT [hd, n]; out = AT.T @ woT -> partial
    [2048, 512] f32 -> DMA out. Projections, transposes and output DMAs
    drip into the exp-paced gaps on an explicit (unit, j-pair) schedule.
"""

import functools
import sys

if "/opt/trn_rl_repo" not in sys.path:
    sys.path.insert(0, "/opt/trn_rl_repo")

import numpy as np
import ml_dtypes

import concourse.bacc as bacc
import concourse.mybir as mybir
import concourse.tile as tile
from concourse.bass_utils import run_bass_kernel_spmd

N_CORES = 8
B = 4
N = 2048          # sequence length
C = 512           # model dim
HPC = 4           # heads per core
D = 64            # head dim
SCALE = D ** -0.5

F32 = mybir.dt.float32
BF16 = mybir.dt.bfloat16

NT = N // 128     # 16 n/j tiles of 128
KT = C // 128     # 4 contraction tiles for the projections
IB = 4            # i-blocks of 512
JP = NT // 2      # 8 j-tile pairs per i-block


def _build_body(nc, tc, ctx, xT_d, wqkT_d, wvT_d, woT_d, ident_d, out_d):
    sb = ctx.enter_context(tc.tile_pool(name="sb", bufs=1))
    work = ctx.enter_context(tc.tile_pool(name="work", bufs=2))
    ptp = ctx.enter_context(tc.tile_pool(name="ptp", bufs=4))
    psp = ctx.enter_context(tc.tile_pool(name="psp", bufs=2, space="PSUM"))
    pup = ctx.enter_context(tc.tile_pool(name="pup", bufs=2, space="PSUM"))
    ppp = ctx.enter_context(tc.tile_pool(name="ppp", bufs=2, space="PSUM"))

    # ---- persistent SBUF tensors ----
    xT = [sb.tile([128, N], BF16, tag=f"x{k}", name=f"x{k}") for k in range(KT)]
    wqk = [sb.tile([128, 512], BF16, tag=f"wqk{k}", name=f"wqk{k}") for k in range(KT)]
    wv = [sb.tile([128, 256], BF16, tag=f"wv{k}", name=f"wv{k}") for k in range(KT)]
    wo = [sb.tile([128, 512], BF16, tag=f"wo{t}", name=f"wo{t}") for t in range(2)]
    ident = sb.tile([128, 128], BF16, tag="ident", name="ident")
    qkT = [sb.tile([128, N], BF16, tag=f"qk{o}", name=f"qk{o}") for o in range(4)]
    vsb = [sb.tile([128, HPC * 65], BF16, tag=f"v{t}", name=f"v{t}") for t in range(NT)]
    Asb = [sb.tile([128, 256], BF16, tag=f"a{t}", name=f"a{t}") for t in range(NT)]
    ATsb = [sb.tile([128, N], BF16, tag=f"at{t}", name=f"at{t}") for t in range(2)]

    # warm the ACT exp table set at t=0 so the one-time ~1.3us table DMA
    # overlaps the input DMAs instead of delaying the first real exp
    warm = work.tile([128, 16], F32, tag="warm", name="warm", bufs=1)
    nc.vector.memset(warm[:], 0.0)
    nc.scalar.activation(warm[:], warm[:], mybir.ActivationFunctionType.Exp)

    # ---- input DMAs: every dma_start fans its per-partition descriptors
    # over all 16 HWDGE queues, so issue ORDER is what prioritizes. wv is
    # tiny (256KB) and feeds unit 0's v fillers -> first; then wqk + xT
    # chunk 0 (the prologue's critical 1.5MB); everything else after ----
    for k in range(KT):
        (nc.sync if k % 2 == 0 else nc.scalar).dma_start(
            out=wv[k][:], in_=wvT_d[k * 128:(k + 1) * 128, :])
    for k in range(KT):
        weng = nc.sync if k % 2 == 0 else nc.scalar
        weng.dma_start(out=wqk[k][:], in_=wqkT_d[k * 128:(k + 1) * 128, :])
        weng.dma_start(out=xT[k][:, 0:512], in_=xT_d[k * 128:(k + 1) * 128, 0:512])
    for nch in range(1, 4):
        for k in range(KT):
            eng = (nc.sync, nc.scalar, nc.gpsimd)[(nch * KT + k) % 3]
            eng.dma_start(
                out=xT[k][:, nch * 512:(nch + 1) * 512],
                in_=xT_d[k * 128:(k + 1) * 128, nch * 512:(nch + 1) * 512],
            )
    for t in range(2):
        nc.gpsimd.dma_start(out=wo[t][:], in_=woT_d[t * 128:(t + 1) * 128, :])
    nc.gpsimd.dma_start(out=ident[:], in_=ident_d[:, :])

    # ones columns of v tiles (never overwritten by the v eviction)
    for t in range(NT):
        v3 = vsb[t][:].rearrange("p (h c) -> p h c", c=65)
        nc.vector.memset(v3[:, :, 64:65], 1.0)

    # ---- filler emitters (dripped into the exp-paced gaps) ----
    # qk o-tiles: 0 = q heads 0/1, 1 = q heads 2/3, 2 = k heads 0/1, 3 = k 2/3.
    def qk_chunk(ot, nch):
        def f():
            pp = ppp.tile([128, 512], F32, tag="pp", name="pp")
            for k in range(KT):
                nc.tensor.matmul(
                    pp[:, 0:512],
                    wqk[k][:, ot * 128:(ot + 1) * 128],
                    xT[k][:, nch * 512:(nch + 1) * 512],
                    start=(k == 0),
                    stop=(k == KT - 1),
                )
            nc.vector.tensor_copy(qkT[ot][:, nch * 512:(nch + 1) * 512], pp[:, 0:512])
        return f

    def v_pair(tp, hp):
        # v projection for head-pair hp (columns hp*128:(hp+1)*128 of wv);
        # both tiles share one PSUM buffer in disjoint regions so the
        # second tile's matmuls don't wait on the first tile's cast
        def f():
            pp = ppp.tile([128, 512], F32, tag="pp", name="ppv")
            for i, t in enumerate((2 * tp, 2 * tp + 1)):
                for k in range(KT):
                    nc.tensor.matmul(
                        pp[:, i * 128:(i + 1) * 128],
                        xT[k][:, t * 128:(t + 1) * 128],
                        wv[k][:, hp * 128:(hp + 1) * 128],
                        start=(k == 0),
                        stop=(k == KT - 1),
                    )
            for i, t in enumerate((2 * tp, 2 * tp + 1)):
                v3 = vsb[t][:].rearrange("p (h c) -> p h c", c=65)
                p3 = pp[:, i * 128:(i + 1) * 128].rearrange("p (h c) -> p h c", c=64)
                nc.vector.tensor_copy(v3[:, 2 * hp:2 * hp + 2, 0:64], p3)
        return f

    def out_transpose(nt):
        def f():
            tp = ppp.tile([128, 512], BF16, tag="pp", name="tp")
            for t2 in range(2):
                nc.tensor.transpose(
                    tp[:, t2 * 128:(t2 + 1) * 128],
                    Asb[nt][:, t2 * 128:(t2 + 1) * 128], ident[:])
            for t2 in range(2):
                nc.vector.tensor_copy(
                    ATsb[t2][:, nt * 128:(nt + 1) * 128],
                    tp[:, t2 * 128:(t2 + 1) * 128])
        return f

    def out_piece(nt, h):
        # per-head transpose piece: [128 i, 64] -> AT rows (h%2)*64
        def f():
            tp = ppp.tile([128, 512], BF16, tag="pp", name="tpp")
            nc.tensor.transpose(
                tp[0:64, 0:128], Asb[nt][:, h * 64:(h + 1) * 64], ident[:])
            nc.vector.tensor_copy(
                ATsb[h // 2][(h % 2) * 64:(h % 2) * 64 + 64,
                             nt * 128:(nt + 1) * 128],
                tp[0:64, 0:128],
            )
        return f

    def out_proj(nt):
        def f():
            ppo = ppp.tile([128, 512], F32, tag="pp", name="ppo")
            for t2 in range(2):
                nc.tensor.matmul(
                    ppo[:, 0:512],
                    ATsb[t2][:, nt * 128:(nt + 1) * 128],
                    wo[t2][:],
                    start=(t2 == 0),
                    stop=(t2 == 1),
                )
            osb = work.tile([128, 512], F32, tag="osb", name="osb")
            nc.vector.tensor_copy(osb[:], ppo[:, 0:512])
            eng = (nc.sync, nc.gpsimd)[nt % 2]
            eng.dma_start(out=out_d[nt * 128:(nt + 1) * 128, :], in_=osb[:])
        return f

    # ---- unit order: head-pair 0/1 over all i-blocks first, then 2/3, so
    # the q/k projections for heads 2/3 drip in long after the fill phase ----
    units = [(ib, hp * 2 + h) for hp in range(2) for ib in range(IB)
             for h in range(2)]

    # ---- filler schedule: (unit, jp) -> closures, emitted right after the
    # exp so the PE chews them while ScalarE works and PV waits its sem ----
    sched = {}

    def at(u, jp, f):
        sched.setdefault((u, jp), []).append(f)

    # NOTE: S(k) is emitted two slots early (at slot k-2), so a qk chunk
    # feeding S(u, jp) must be scheduled strictly before slot (u, jp-2).
    for jp in range(JP):
        at(0, jp, v_pair(jp, 0))              # h0/h1 V, consumed by u0's PV
    at(0, 1, qk_chunk(2, 2))                  # k h0/1 chunk 2, S emitted at (0,2)
    at(0, 3, qk_chunk(2, 3))                  # k h0/1 chunk 3, S emitted at (0,4)
    at(1, 0, qk_chunk(0, 1))                  # q h0/1 chunk 1, due u2
    at(3, 0, qk_chunk(0, 2))                  # due u4
    at(5, 0, qk_chunk(0, 3))                  # due u6
    for tp in range(4):
        at(4, 1 + tp, v_pair(tp, 1))          # h2/h3 V, due u8
        at(5, 1 + tp, v_pair(4 + tp, 1))
    at(5, 5, qk_chunk(3, 0))                  # k h2/3, due u8
    at(6, 2, qk_chunk(3, 1))
    at(6, 5, qk_chunk(3, 2))                  # due u8.jp4
    at(7, 5, qk_chunk(3, 3))                  # due u8.jp6
    at(7, 4, qk_chunk(1, 0))                  # q h2/3 chunk 0, S(u8,0) at (7,6)
    at(9, 0, qk_chunk(1, 1))                  # due u10
    at(11, 0, qk_chunk(1, 2))                 # due u12
    at(13, 0, qk_chunk(1, 3))                 # due u14
    # group-3 A slices transpose per head as soon as each head finishes
    # (units 6, 7, 14 for h0, h1, h2); h3 lands in the tail
    for c in range(4):
        at(7, c, out_piece(12 + c, 0))
        at(8, c, out_piece(12 + c, 1))
        at(15, c, out_piece(12 + c, 2))
    # groups 0-2 complete at units 9, 11, 13; whole-tile transposes + proj
    for g in range(3):
        for c in range(4):
            at(10 + 2 * g, c, out_transpose(4 * g + c))
        for c in range(3):
            at(10 + 2 * g, 4 + c, out_proj(4 * g + c))
        at(11 + 2 * g, 4, out_proj(4 * g + 3))

    # ---- main pipeline, software-pipelined two slots deep ----
    exp_t = mybir.ActivationFunctionType.Exp

    slots = [(ui, jp) for ui in range(len(units)) for jp in range(JP)]
    ps_tiles = {}

    def emit_S(k):
        ui, jp = slots[k]
        ib, h = units[ui]
        rows = slice((h % 2) * 64, (h % 2) * 64 + 64)
        ps = psp.tile([128, 1024], F32, tag="ps", name="ps")
        for half in range(2):
            jt = jp * 2 + half
            nc.tensor.matmul(
                ps[:, half * 512:(half + 1) * 512],
                qkT[2 + h // 2][rows, jt * 128:(jt + 1) * 128],
                qkT[h // 2][rows, ib * 512:(ib + 1) * 512],
                start=True,
                stop=True,
            )
        ps_tiles[k] = ps

    qk_chunk(2, 0)()
    qk_chunk(0, 0)()
    emit_S(0)
    emit_S(1)
    qk_chunk(2, 1)()                          # k h0/1 chunk 1, before S(0,2)
                                              # which is emitted at slot (0,0)

    pu = None
    for k, (ui, jp) in enumerate(slots):
        ib, h = units[ui]
        ps = ps_tiles.pop(k)
        pt = ptp.tile([128, 1024], BF16, tag="pt", name="pt")
        nc.scalar.activation(pt[:], ps[:], exp_t, scale=SCALE)
        if k + 2 < len(slots):
            emit_S(k + 2)
        for f in sched.get((ui, jp), ()):
            f()
        if jp == 0:
            pu = pup.tile([128, 260], F32, tag="pu", name="pu")
        for half in range(2):
            jt = jp * 2 + half
            for c in range(4):
                # start=True clears the whole PSUM bank, so only the very
                # first matmul of the unit sets it; the other i-chunks'
                # first writes rely on per-element has_written overwrite.
                nc.tensor.matmul(
                    pu[:, c * 65:(c + 1) * 65],
                    pt[:, half * 512 + c * 128:half * 512 + (c + 1) * 128],
                    vsb[jt][:, h * 65:(h + 1) * 65],
                    start=(jt == 0 and c == 0),
                    stop=(jt == NT - 1),
                    skip_group_check=True,
                )
        if jp == JP - 1:
            for c in range(4):
                rec = work.tile([128, 1], F32, tag="rec", name="rec")
                nc.vector.reciprocal(rec[:], pu[:, c * 65 + 64:c * 65 + 65])
                nc.vector.tensor_scalar_mul(
                    Asb[ib * 4 + c][:, h * 64:(h + 1) * 64],
                    pu[:, c * 65:c * 65 + 64],
                    rec[:],
                )
    # tail: interleave the h3 transpose pieces with the projections so the
    # PE works while the DVE casts drain
    out_piece(12, 3)()
    out_piece(13, 3)()
    out_proj(12)()
    out_piece(14, 3)()
    out_proj(13)()
    out_piece(15, 3)()
    out_proj(14)()
    out_proj(15)()


@functools.lru_cache(maxsize=1)
def _build():
    nc = bacc.Bacc("TRN2", target_bir_lowering=False, debug=False,
                   num_devices=N_CORES)
    xT_d = nc.dram_tensor("xT", [C, N], BF16, kind="ExternalInput").ap()
    wqkT_d = nc.dram_tensor("wqkT", [C, 512], BF16, kind="ExternalInput").ap()
    wvT_d = nc.dram_tensor("wvT", [C, 256], BF16, kind="ExternalInput").ap()
    woT_d = nc.dram_tensor("woT", [256, C], BF16, kind="ExternalInput").ap()
    ident_d = nc.dram_tensor("ident", [128, 128], BF16, kind="ExternalInput").ap()
    out_d = nc.dram_tensor("out", [N, C], F32, kind="ExternalOutput").ap()
    from contextlib import ExitStack
    with tile.TileContext(nc) as tc, ExitStack() as ctx:
        _build_body(nc, tc, ctx, xT_d, wqkT_d, wvT_d, woT_d, ident_d, out_d)
    nc.compile()
    return nc


def _shard_inputs(x, W_qkv, W_out):
    bf16 = ml_dtypes.bfloat16
    ident = np.eye(128, dtype=bf16)
    in_maps = []
    for core in range(N_CORES):
        b, hg = core // 2, core % 2
        xT = np.ascontiguousarray(x[b].T).astype(bf16)
        rows_q = W_qkv[hg * 256:(hg + 1) * 256, :]
        rows_k = W_qkv[512 + hg * 256:512 + (hg + 1) * 256, :]
        wqkT = np.ascontiguousarray(
            np.concatenate([rows_q, rows_k], 0).T).astype(bf16)
        wvT = np.ascontiguousarray(
            W_qkv[1024 + hg * 256:1024 + (hg + 1) * 256, :].T).astype(bf16)
        woT = np.ascontiguousarray(
            W_out[:, hg * 256:(hg + 1) * 256].T).astype(bf16)
        in_maps.append(
            {"xT": xT, "wqkT": wqkT, "wvT": wvT, "woT": woT, "ident": ident})
    return in_maps


def _run(inputs, trace=False, tmpdir=None):
    x = np.asarray(inputs["x"], dtype=np.float32)
    W_qkv = np.asarray(inputs["W_qkv"], dtype=np.float32)
    W_out = np.asarray(inputs["W_out"], dtype=np.float32)
    nc = _build()
    in_maps = _shard_inputs(x, W_qkv, W_out)
    kwargs = {}
    if trace:
        kwargs = dict(trace=True, tmpdir=tmpdir)
    res = run_bass_kernel_spmd(nc, in_maps, core_ids=list(range(N_CORES)), **kwargs)
    out = np.zeros((B, N, C), np.float32)
    for core in range(N_CORES):
        out[core // 2] += res.results[core]["out"]
    return out, res


def kernel(**inputs):
    out, _ = _run(inputs)
    return out


# revision 27
# speedup vs baseline: 1.0809x; 1.0027x over previous
"""Multi-head attention (b=4, n=2048, dim=512, heads=8, d_head=64) on 8 TRN2 NeuronCores.

Sharding: core = 2*b + head_group. Data parallel over batch (4), tensor
parallel over heads (2 groups of 4). Each core computes the QKV projection
for its 4 heads, full attention, and a partial output projection (its
heads' rows of W_out); the host sums the two partials per batch.

Device pipeline per core, engine-balanced so the ScalarE exp stream
(~8.7us per (i-block, head) unit) paces everything while PE/DVE hide
under it:
  - qk^T = wqkT.T @ xT -> [512(o), 2048(n)] (partition dim = head-major d)
  - v = xT.T @ wvT -> [2048, 256], stored as [128, 4*65] tiles with a 1.0
    column per head so the PV matmul also produces the softmax denominator
  - per slot (unit, j-pair): S^T[j,i] = k^T.T @ q^T into PSUM; P~ =
    exp(S^T * scale) on ScalarE; PV *flipped*: U[i,65] += P~[j,i].T @
    [v_h|1][j,65] -- output partitions are i, so the denominator is a
    per-partition scalar (cheap batched reciprocal + tensor_scalar).
  - scores for slot k+2 are emitted at slot k (ps pool bufs=3), so ScalarE
    holds a two-slot lead and never starves while the PE chews fillers.
  - per n-tile: PE-transpose A -> AT [hd, n]; out = AT.T @ woT -> partial
    [2048, 512] f32 -> DMA out. Projections, transposes and output DMAs
    drip into the exp-paced gaps on an explicit (unit, j-pair) schedule.
"""

import functools
import sys

if "/opt/trn_rl_repo" not in sys.path:
    sys.path.insert(0, "/opt/trn_rl_repo")

import numpy as np
import ml_dtypes

import concourse.bacc as bacc
import concourse.mybir as mybir
import concourse.tile as tile
from concourse.bass_utils import run_bass_kernel_spmd

N_CORES = 8
B = 4
N = 2048          # sequence length
C = 512           # model dim
HPC = 4           # heads per core
D = 64            # head dim
SCALE = D ** -0.5

F32 = mybir.dt.float32
BF16 = mybir.dt.bfloat16

NT = N // 128     # 16 n/j tiles of 128
KT = C // 128     # 4 contraction tiles for the projections
IB = 4            # i-blocks of 512
JP = NT // 2      # 8 j-tile pairs per i-block


def _build_body(nc, tc, ctx, xT_d, wqkT_d, wvT_d, woT_d, ident_d, out_d):
    sb = ctx.enter_context(tc.tile_pool(name="sb", bufs=1))
    work = ctx.enter_context(tc.tile_pool(name="work", bufs=2))
    ptp = ctx.enter_context(tc.tile_pool(name="ptp", bufs=4))
    psp = ctx.enter_context(tc.tile_pool(name="psp", bufs=2, space="PSUM"))
    pup = ctx.enter_context(tc.tile_pool(name="pup", bufs=2, space="PSUM"))
    ppp = ctx.enter_context(tc.tile_pool(name="ppp", bufs=2, space="PSUM"))

    # ---- persistent SBUF tensors ----
    xT = [sb.tile([128, N], BF16, tag=f"x{k}", name=f"x{k}") for k in range(KT)]
    wqk = [sb.tile([128, 512], BF16, tag=f"wqk{k}", name=f"wqk{k}") for k in range(KT)]
    wv = [sb.tile([128, 256], BF16, tag=f"wv{k}", name=f"wv{k}") for k in range(KT)]
    wo = [sb.tile([128, 512], BF16, tag=f"wo{t}", name=f"wo{t}") for t in range(2)]
    ident = sb.tile([128, 128], BF16, tag="ident", name="ident")
    qkT = [sb.tile([128, N], BF16, tag=f"qk{o}", name=f"qk{o}") for o in range(4)]
    vsb = [sb.tile([128, HPC * 65], BF16, tag=f"v{t}", name=f"v{t}") for t in range(NT)]
    Asb = [sb.tile([128, 256], BF16, tag=f"a{t}", name=f"a{t}") for t in range(NT)]
    ATsb = [sb.tile([128, N], BF16, tag=f"at{t}", name=f"at{t}") for t in range(2)]

    # ---- input DMAs: every dma_start fans its per-partition descriptors
    # over all 16 HWDGE queues, so issue ORDER prioritizes. Only sync and
    # gpsimd issue the critical transfers -- the scalar sequencer must stay
    # free: anything queued there would sit behind the ACT table load and
    # delay its completion semaphores. wv (tiny, feeds unit 0's v fillers)
    # first, then wqk + xT chunk 0 (the prologue's 1.5MB), then the rest ----
    for k in range(KT):
        (nc.sync if k % 2 == 0 else nc.gpsimd).dma_start(
            out=wv[k][:], in_=wvT_d[k * 128:(k + 1) * 128, :])
    for k in range(KT):
        weng = nc.sync if k % 2 == 0 else nc.gpsimd
        weng.dma_start(out=wqk[k][:], in_=wqkT_d[k * 128:(k + 1) * 128, :])
        weng.dma_start(out=xT[k][:, 0:512], in_=xT_d[k * 128:(k + 1) * 128, 0:512])
    for k in range(KT):
        eng = nc.sync if k % 2 == 0 else nc.gpsimd
        eng.dma_start(out=xT[k][:, 512:2048], in_=xT_d[k * 128:(k + 1) * 128, 512:2048])
    for t in range(2):
        nc.gpsimd.dma_start(out=wo[t][:], in_=woT_d[t * 128:(t + 1) * 128, :])
    nc.gpsimd.dma_start(out=ident[:], in_=ident_d[:, :])

    # warm the ACT exp table set (one-time ~1.3us table DMA) while the
    # input DMAs stream; emitted after the triggers so it delays nothing
    warm = work.tile([128, 16], F32, tag="warm", name="warm", bufs=1)
    nc.vector.memset(warm[:], 0.0)
    nc.scalar.activation(warm[:], warm[:], mybir.ActivationFunctionType.Exp)

    # ones columns of v tiles (never overwritten by the v eviction)
    for t in range(NT):
        v3 = vsb[t][:].rearrange("p (h c) -> p h c", c=65)
        nc.vector.memset(v3[:, :, 64:65], 1.0)

    # ---- filler emitters (dripped into the exp-paced gaps) ----
    # qk o-tiles: 0 = q heads 0/1, 1 = q heads 2/3, 2 = k heads 0/1, 3 = k 2/3.
    def qk_chunk(ot, nch):
        def f():
            pp = ppp.tile([128, 512], F32, tag="pp", name="pp")
            for k in range(KT):
                nc.tensor.matmul(
                    pp[:, 0:512],
                    wqk[k][:, ot * 128:(ot + 1) * 128],
                    xT[k][:, nch * 512:(nch + 1) * 512],
                    start=(k == 0),
                    stop=(k == KT - 1),
                )
            nc.vector.tensor_copy(qkT[ot][:, nch * 512:(nch + 1) * 512], pp[:, 0:512])
        return f

    def v_pair(tp, hp):
        # v projection for head-pair hp (columns hp*128:(hp+1)*128 of wv);
        # both tiles share one PSUM buffer in disjoint regions so the
        # second tile's matmuls don't wait on the first tile's cast
        def f():
            pp = ppp.tile([128, 512], F32, tag="pp", name="ppv")
            for i, t in enumerate((2 * tp, 2 * tp + 1)):
                for k in range(KT):
                    nc.tensor.matmul(
                        pp[:, i * 128:(i + 1) * 128],
                        xT[k][:, t * 128:(t + 1) * 128],
                        wv[k][:, hp * 128:(hp + 1) * 128],
                        start=(k == 0),
                        stop=(k == KT - 1),
                    )
            for i, t in enumerate((2 * tp, 2 * tp + 1)):
                v3 = vsb[t][:].rearrange("p (h c) -> p h c", c=65)
                p3 = pp[:, i * 128:(i + 1) * 128].rearrange("p (h c) -> p h c", c=64)
                nc.vector.tensor_copy(v3[:, 2 * hp:2 * hp + 2, 0:64], p3)
        return f

    def out_transpose(nt):
        def f():
            tp = ppp.tile([128, 512], BF16, tag="pp", name="tp")
            for t2 in range(2):
                nc.tensor.transpose(
                    tp[:, t2 * 128:(t2 + 1) * 128],
                    Asb[nt][:, t2 * 128:(t2 + 1) * 128], ident[:])
            for t2 in range(2):
                nc.vector.tensor_copy(
                    ATsb[t2][:, nt * 128:(nt + 1) * 128],
                    tp[:, t2 * 128:(t2 + 1) * 128])
        return f

    def out_piece(nt, h):
        # per-head transpose piece: [128 i, 64] -> AT rows (h%2)*64
        def f():
            tp = ppp.tile([128, 512], BF16, tag="pp", name="tpp")
            nc.tensor.transpose(
                tp[0:64, 0:128], Asb[nt][:, h * 64:(h + 1) * 64], ident[:])
            nc.vector.tensor_copy(
                ATsb[h // 2][(h % 2) * 64:(h % 2) * 64 + 64,
                             nt * 128:(nt + 1) * 128],
                tp[0:64, 0:128],
            )
        return f

    def out_proj(nt):
        def f():
            ppo = ppp.tile([128, 512], F32, tag="pp", name="ppo")
            for t2 in range(2):
                nc.tensor.matmul(
                    ppo[:, 0:512],
                    ATsb[t2][:, nt * 128:(nt + 1) * 128],
                    wo[t2][:],
                    start=(t2 == 0),
                    stop=(t2 == 1),
                )
            osb = work.tile([128, 512], F32, tag="osb", name="osb")
            nc.vector.tensor_copy(osb[:], ppo[:, 0:512])
            eng = (nc.sync, nc.gpsimd)[nt % 2]
            eng.dma_start(out=out_d[nt * 128:(nt + 1) * 128, :], in_=osb[:])
        return f

    # ---- unit order: head-pair 0/1 over all i-blocks first, then 2/3, so
    # the q/k projections for heads 2/3 drip in long after the fill phase ----
    units = [(ib, hp * 2 + h) for hp in range(2) for ib in range(IB)
             for h in range(2)]

    # ---- filler schedule: (unit, jp) -> closures, emitted right after the
    # exp so the PE chews them while ScalarE works and PV waits its sem ----
    sched = {}

    def at(u, jp, f):
        sched.setdefault((u, jp), []).append(f)

    # NOTE: S(k) is emitted two slots early (at slot k-2), so a qk chunk
    # feeding S(u, jp) must be scheduled strictly before slot (u, jp-2).
    for jp in range(JP):
        at(0, jp, v_pair(jp, 0))              # h0/h1 V, consumed by u0's PV
    at(0, 1, qk_chunk(2, 2))                  # k h0/1 chunk 2, S emitted at (0,2)
    at(0, 3, qk_chunk(2, 3))                  # k h0/1 chunk 3, S emitted at (0,4)
    at(1, 0, qk_chunk(0, 1))                  # q h0/1 chunk 1, due u2
    at(3, 0, qk_chunk(0, 2))                  # due u4
    at(5, 0, qk_chunk(0, 3))                  # due u6
    for tp in range(4):
        at(4, 1 + tp, v_pair(tp, 1))          # h2/h3 V, due u8
        at(5, 1 + tp, v_pair(4 + tp, 1))
    at(5, 5, qk_chunk(3, 0))                  # k h2/3, due u8
    at(6, 2, qk_chunk(3, 1))
    at(6, 5, qk_chunk(3, 2))                  # due u8.jp4
    at(7, 5, qk_chunk(3, 3))                  # due u8.jp6
    at(7, 4, qk_chunk(1, 0))                  # q h2/3 chunk 0, S(u8,0) at (7,6)
    at(9, 0, qk_chunk(1, 1))                  # due u10
    at(11, 0, qk_chunk(1, 2))                 # due u12
    at(13, 0, qk_chunk(1, 3))                 # due u14
    # group-3 A slices transpose per head as soon as each head finishes
    # (units 6, 7, 14 for h0, h1, h2); h3 lands in the tail
    for c in range(4):
        at(7, c, out_piece(12 + c, 0))
        at(8, c, out_piece(12 + c, 1))
        at(15, c, out_piece(12 + c, 2))
    # groups 0-2 complete at units 9, 11, 13; whole-tile transposes + proj
    for g in range(3):
        for c in range(4):
            at(10 + 2 * g, c, out_transpose(4 * g + c))
        for c in range(3):
            at(10 + 2 * g, 4 + c, out_proj(4 * g + c))
        at(11 + 2 * g, 4, out_proj(4 * g + 3))

    # ---- main pipeline, software-pipelined two slots deep ----
    exp_t = mybir.ActivationFunctionType.Exp

    slots = [(ui, jp) for ui in range(len(units)) for jp in range(JP)]
    ps_tiles = {}

    def emit_S(k):
        ui, jp = slots[k]
        ib, h = units[ui]
        rows = slice((h % 2) * 64, (h % 2) * 64 + 64)
        ps = psp.tile([128, 1024], F32, tag="ps", name="ps")
        for half in range(2):
            jt = jp * 2 + half
            nc.tensor.matmul(
                ps[:, half * 512:(half + 1) * 512],
                qkT[2 + h // 2][rows, jt * 128:(jt + 1) * 128],
                qkT[h // 2][rows, ib * 512:(ib + 1) * 512],
                start=True,
                stop=True,
            )
        ps_tiles[k] = ps

    qk_chunk(2, 0)()
    qk_chunk(0, 0)()
    emit_S(0)
    emit_S(1)
    qk_chunk(2, 1)()                          # k h0/1 chunk 1, before S(0,2)
                                              # which is emitted at slot (0,0)

    pu = None
    for k, (ui, jp) in enumerate(slots):
        ib, h = units[ui]
        ps = ps_tiles.pop(k)
        pt = ptp.tile([128, 1024], BF16, tag="pt", name="pt")
        nc.scalar.activation(pt[:], ps[:], exp_t, scale=SCALE)
        if k + 2 < len(slots):
            emit_S(k + 2)
        for f in sched.get((ui, jp), ()):
            f()
        if jp == 0:
            pu = pup.tile([128, 260], F32, tag="pu", name="pu")
        for half in range(2):
            jt = jp * 2 + half
            for c in range(4):
                # start=True clears the whole PSUM bank, so only the very
                # first matmul of the unit sets it; the other i-chunks'
                # first writes rely on per-element has_written overwrite.
                nc.tensor.matmul(
                    pu[:, c * 65:(c + 1) * 65],
                    pt[:, half * 512 + c * 128:half * 512 + (c + 1) * 128],
                    vsb[jt][:, h * 65:(h + 1) * 65],
                    start=(jt == 0 and c == 0),
                    stop=(jt == NT - 1),
                    skip_group_check=True,
                )
        if jp == JP - 1:
            for c in range(4):
                rec = work.tile([128, 1], F32, tag="rec", name="rec")
                nc.vector.reciprocal(rec[:], pu[:, c * 65 + 64:c * 65 + 65])
                nc.vector.tensor_scalar_mul(
                    Asb[ib * 4 + c][:, h * 64:(h + 1) * 64],
                    pu[:, c * 65:c * 65 + 64],
                    rec[:],
                )
    # tail: interleave the h3 transpose pieces with the projections so the
    # PE works while the DVE casts drain
    out_piece(12, 3)()
    out_piece(13, 3)()
    out_proj(12)()
    out_piece(14, 3)()
    out_proj(13)()
    out_piece(15, 3)()
    out_proj(14)()
    out_proj(15)()


@functools.lru_cache(maxsize=1)
def _build():
    nc = bacc.Bacc("TRN2", target_bir_lowering=False, debug=False,
                   num_devices=N_CORES)
    xT_d = nc.dram_tensor("xT", [C, N], BF16, kind="ExternalInput").ap()
    wqkT_d = nc.dram_tensor("wqkT", [C, 512], BF16, kind="ExternalInput").ap()
    wvT_d = nc.dram_tensor("wvT", [C, 256], BF16, kind="ExternalInput").ap()
    woT_d = nc.dram_tensor("woT", [256, C], BF16, kind="ExternalInput").ap()
    ident_d = nc.dram_tensor("ident", [128, 128], BF16, kind="ExternalInput").ap()
    out_d = nc.dram_tensor("out", [N, C], F32, kind="ExternalOutput").ap()
    from contextlib import ExitStack
    with tile.TileContext(nc) as tc, ExitStack() as ctx:
        _build_body(nc, tc, ctx, xT_d, wqkT_d, wvT_d, woT_d, ident_d, out_d)
    nc.compile()
    return nc


def _shard_inputs(x, W_qkv, W_out):
    bf16 = ml_dtypes.bfloat16
    ident = np.eye(128, dtype=bf16)
    in_maps = []
    for core in range(N_CORES):
        b, hg = core // 2, core % 2
        xT = np.ascontiguousarray(x[b].T).astype(bf16)
        rows_q = W_qkv[hg * 256:(hg + 1) * 256, :]
        rows_k = W_qkv[512 + hg * 256:512 + (hg + 1) * 256, :]
        wqkT = np.ascontiguousarray(
            np.concatenate([rows_q, rows_k], 0).T).astype(bf16)
        wvT = np.ascontiguousarray(
            W_qkv[1024 + hg * 256:1024 + (hg + 1) * 256, :].T).astype(bf16)
        woT = np.ascontiguousarray(
            W_out[:, hg * 256:(hg + 1) * 256].T).astype(bf16)
        in_maps.append(
            {"xT": xT, "wqkT": wqkT, "wvT": wvT, "woT": woT, "ident": ident})
    return in_maps


def _run(inputs, trace=False, tmpdir=None):
    x = np.asarray(inputs["x"], dtype=np.float32)
    W_qkv = np.asarray(inputs["W_qkv"], dtype=np.float32)
    W_out = np.asarray(inputs["W_out"], dtype=np.float32)
    nc = _build()
    in_maps = _shard_inputs(x, W_qkv, W_out)
    kwargs = {}
    if trace:
        kwargs = dict(trace=True, tmpdir=tmpdir)
    res = run_bass_kernel_spmd(nc, in_maps, core_ids=list(range(N_CORES)), **kwargs)
    out = np.zeros((B, N, C), np.float32)
    for core in range(N_CORES):
        out[core // 2] += res.results[core]["out"]
    return out, res


def kernel(**inputs):
    out, _ = _run(inputs)
    return out


# revision 29
# speedup vs baseline: 1.0853x; 1.0040x over previous
"""Multi-head attention (b=4, n=2048, dim=512, heads=8, d_head=64) on 8 TRN2 NeuronCores.

Sharding: core = 2*b + head_group. Data parallel over batch (4), tensor
parallel over heads (2 groups of 4). Each core computes the QKV projection
for its 4 heads, full attention, and a partial output projection (its
heads' rows of W_out); the host sums the two partials per batch.

Device pipeline per core, engine-balanced so the ScalarE exp stream
(~8.7us per (i-block, head) unit) paces everything while PE/DVE hide
under it:
  - qk^T = wqkT.T @ xT -> [512(o), 2048(n)] (partition dim = head-major d)
  - v = xT.T @ wvT -> [2048, 256], stored as [128, 4*65] tiles with a 1.0
    column per head so the PV matmul also produces the softmax denominator
  - per slot (unit, j-pair): S^T[j,i] = k^T.T @ q^T into PSUM; P~ =
    exp(S^T * scale) on ScalarE; PV *flipped*: U[i,65] += P~[j,i].T @
    [v_h|1][j,65] -- output partitions are i, so the denominator is a
    per-partition scalar (cheap batched reciprocal + tensor_scalar).
  - scores for slot k+2 are emitted at slot k (ps pool bufs=3), so ScalarE
    holds a two-slot lead and never starves while the PE chews fillers.
  - per n-tile: PE-transpose A -> AT [hd, n]; out = AT.T @ woT -> partial
    [2048, 512] f32 -> DMA out. Projections, transposes and output DMAs
    drip into the exp-paced gaps on an explicit (unit, j-pair) schedule.
"""

import functools
import sys

if "/opt/trn_rl_repo" not in sys.path:
    sys.path.insert(0, "/opt/trn_rl_repo")

import numpy as np
import ml_dtypes

import concourse.bacc as bacc
import concourse.mybir as mybir
import concourse.tile as tile
from concourse.bass_utils import run_bass_kernel_spmd

N_CORES = 8
B = 4
N = 2048          # sequence length
C = 512           # model dim
HPC = 4           # heads per core
D = 64            # head dim
SCALE = D ** -0.5

F32 = mybir.dt.float32
BF16 = mybir.dt.bfloat16

NT = N // 128     # 16 n/j tiles of 128
KT = C // 128     # 4 contraction tiles for the projections
IB = 4            # i-blocks of 512
JP = NT // 2      # 8 j-tile pairs per i-block


def _build_body(nc, tc, ctx, xT_d, wqkT_d, wvT_d, woT_d, ident_d, out_d):
    sb = ctx.enter_context(tc.tile_pool(name="sb", bufs=1))
    work = ctx.enter_context(tc.tile_pool(name="work", bufs=2))
    ptp = ctx.enter_context(tc.tile_pool(name="ptp", bufs=4))
    psp = ctx.enter_context(tc.tile_pool(name="psp", bufs=2, space="PSUM"))
    pup = ctx.enter_context(tc.tile_pool(name="pup", bufs=2, space="PSUM"))
    ppp = ctx.enter_context(tc.tile_pool(name="ppp", bufs=2, space="PSUM"))

    # ---- persistent SBUF tensors ----
    xT = [sb.tile([128, N], BF16, tag=f"x{k}", name=f"x{k}") for k in range(KT)]
    wqk = [sb.tile([128, 512], BF16, tag=f"wqk{k}", name=f"wqk{k}") for k in range(KT)]
    wv = [sb.tile([128, 256], BF16, tag=f"wv{k}", name=f"wv{k}") for k in range(KT)]
    wo = [sb.tile([128, 512], BF16, tag=f"wo{t}", name=f"wo{t}") for t in range(2)]
    ident = sb.tile([128, 128], BF16, tag="ident", name="ident")
    qkT = [sb.tile([128, N], BF16, tag=f"qk{o}", name=f"qk{o}") for o in range(4)]
    vsb = [sb.tile([128, HPC * 65], BF16, tag=f"v{t}", name=f"v{t}") for t in range(NT)]
    Asb = [sb.tile([128, 256], BF16, tag=f"a{t}", name=f"a{t}") for t in range(NT)]
    ATsb = [sb.tile([128, N], BF16, tag=f"at{t}", name=f"at{t}") for t in range(2)]

    # ---- input DMAs: every dma_start fans its per-partition descriptors
    # over all 16 HWDGE queues, so issue ORDER prioritizes. Only sync and
    # gpsimd issue the critical transfers -- the scalar sequencer must stay
    # free: anything queued there would sit behind the ACT table load and
    # delay its completion semaphores. wv (tiny, feeds unit 0's v fillers)
    # first, then wqk + xT chunk 0 (the prologue's 1.5MB), then the rest ----
    for k in range(KT):
        (nc.sync if k % 2 == 0 else nc.scalar).dma_start(
            out=wv[k][:], in_=wvT_d[k * 128:(k + 1) * 128, :])
    for k in range(KT):
        weng = nc.sync if k % 2 == 0 else nc.scalar
        weng.dma_start(out=wqk[k][:], in_=wqkT_d[k * 128:(k + 1) * 128, :])
        weng.dma_start(out=xT[k][:, 0:512], in_=xT_d[k * 128:(k + 1) * 128, 0:512])
    for k in range(KT):
        eng = nc.sync if k % 2 == 0 else nc.scalar
        eng.dma_start(out=xT[k][:, 512:2048], in_=xT_d[k * 128:(k + 1) * 128, 512:2048])
    for t in range(2):
        nc.sync.dma_start(out=wo[t][:], in_=woT_d[t * 128:(t + 1) * 128, :])
    nc.scalar.dma_start(out=ident[:], in_=ident_d[:, :])

    # warm the ACT exp table set (one-time ~1.3us table DMA) while the
    # input DMAs stream; emitted after the triggers so it delays nothing
    warm = work.tile([128, 16], F32, tag="warm", name="warm", bufs=1)
    nc.vector.memset(warm[:], 0.0)
    nc.scalar.activation(warm[:], warm[:], mybir.ActivationFunctionType.Exp)

    # ones columns of v tiles (never overwritten by the v eviction)
    for t in range(NT):
        v3 = vsb[t][:].rearrange("p (h c) -> p h c", c=65)
        nc.vector.memset(v3[:, :, 64:65], 1.0)

    # ---- filler emitters (dripped into the exp-paced gaps) ----
    # qk o-tiles: 0 = q heads 0/1, 1 = q heads 2/3, 2 = k heads 0/1, 3 = k 2/3.
    def qk_chunk(ot, nch):
        def f():
            pp = ppp.tile([128, 512], F32, tag="pp", name="pp")
            for k in range(KT):
                nc.tensor.matmul(
                    pp[:, 0:512],
                    wqk[k][:, ot * 128:(ot + 1) * 128],
                    xT[k][:, nch * 512:(nch + 1) * 512],
                    start=(k == 0),
                    stop=(k == KT - 1),
                )
            nc.vector.tensor_copy(qkT[ot][:, nch * 512:(nch + 1) * 512], pp[:, 0:512])
        return f

    def v_pair(tp, hp):
        # v projection for head-pair hp (columns hp*128:(hp+1)*128 of wv);
        # both tiles share one PSUM buffer in disjoint regions so the
        # second tile's matmuls don't wait on the first tile's cast
        def f():
            pp = ppp.tile([128, 512], F32, tag="pp", name="ppv")
            for i, t in enumerate((2 * tp, 2 * tp + 1)):
                for k in range(KT):
                    nc.tensor.matmul(
                        pp[:, i * 128:(i + 1) * 128],
                        xT[k][:, t * 128:(t + 1) * 128],
                        wv[k][:, hp * 128:(hp + 1) * 128],
                        start=(k == 0),
                        stop=(k == KT - 1),
                    )
            for i, t in enumerate((2 * tp, 2 * tp + 1)):
                v3 = vsb[t][:].rearrange("p (h c) -> p h c", c=65)
                p3 = pp[:, i * 128:(i + 1) * 128].rearrange("p (h c) -> p h c", c=64)
                nc.vector.tensor_copy(v3[:, 2 * hp:2 * hp + 2, 0:64], p3)
        return f

    def out_transpose(nt):
        def f():
            tp = ppp.tile([128, 512], BF16, tag="pp", name="tp")
            for t2 in range(2):
                nc.tensor.transpose(
                    tp[:, t2 * 128:(t2 + 1) * 128],
                    Asb[nt][:, t2 * 128:(t2 + 1) * 128], ident[:])
            for t2 in range(2):
                nc.vector.tensor_copy(
                    ATsb[t2][:, nt * 128:(nt + 1) * 128],
                    tp[:, t2 * 128:(t2 + 1) * 128])
        return f

    def out_piece(nt, h):
        # per-head transpose piece: [128 i, 64] -> AT rows (h%2)*64
        def f():
            tp = ppp.tile([128, 512], BF16, tag="pp", name="tpp")
            nc.tensor.transpose(
                tp[0:64, 0:128], Asb[nt][:, h * 64:(h + 1) * 64], ident[:])
            nc.vector.tensor_copy(
                ATsb[h // 2][(h % 2) * 64:(h % 2) * 64 + 64,
                             nt * 128:(nt + 1) * 128],
                tp[0:64, 0:128],
            )
        return f

    def out_proj(nt):
        def f():
            ppo = ppp.tile([128, 512], F32, tag="pp", name="ppo")
            for t2 in range(2):
                nc.tensor.matmul(
                    ppo[:, 0:512],
                    ATsb[t2][:, nt * 128:(nt + 1) * 128],
                    wo[t2][:],
                    start=(t2 == 0),
                    stop=(t2 == 1),
                )
            osb = work.tile([128, 512], F32, tag="osb", name="osb")
            nc.vector.tensor_copy(osb[:], ppo[:, 0:512])
            eng = (nc.sync, nc.scalar)[nt % 2]
            eng.dma_start(out=out_d[nt * 128:(nt + 1) * 128, :], in_=osb[:])
        return f

    # ---- unit order: head-pair 0/1 over all i-blocks first, then 2/3, so
    # the q/k projections for heads 2/3 drip in long after the fill phase ----
    units = [(ib, hp * 2 + h) for hp in range(2) for ib in range(IB)
             for h in range(2)]

    # ---- filler schedule: (unit, jp) -> closures, emitted right after the
    # exp so the PE chews them while ScalarE works and PV waits its sem ----
    sched = {}

    def at(u, jp, f):
        sched.setdefault((u, jp), []).append(f)

    # NOTE: S(k) is emitted two slots early (at slot k-2), so a qk chunk
    # feeding S(u, jp) must be scheduled strictly before slot (u, jp-2).
    for jp in range(JP):
        at(0, jp, v_pair(jp, 0))              # h0/h1 V, consumed by u0's PV
    at(0, 1, qk_chunk(2, 2))                  # k h0/1 chunk 2, S emitted at (0,2)
    at(0, 3, qk_chunk(2, 3))                  # k h0/1 chunk 3, S emitted at (0,4)
    at(1, 0, qk_chunk(0, 1))                  # q h0/1 chunk 1, due u2
    at(3, 0, qk_chunk(0, 2))                  # due u4
    at(5, 0, qk_chunk(0, 3))                  # due u6
    for tp in range(4):
        at(4, 1 + tp, v_pair(tp, 1))          # h2/h3 V, due u8
        at(5, 1 + tp, v_pair(4 + tp, 1))
    at(5, 5, qk_chunk(3, 0))                  # k h2/3, due u8
    at(6, 2, qk_chunk(3, 1))
    at(6, 5, qk_chunk(3, 2))                  # due u8.jp4
    at(7, 5, qk_chunk(3, 3))                  # due u8.jp6
    at(7, 4, qk_chunk(1, 0))                  # q h2/3 chunk 0, S(u8,0) at (7,6)
    at(9, 0, qk_chunk(1, 1))                  # due u10
    at(11, 0, qk_chunk(1, 2))                 # due u12
    at(13, 0, qk_chunk(1, 3))                 # due u14
    # group-3 A slices transpose per head as soon as each head finishes
    # (units 6, 7, 14 for h0, h1, h2); h3 lands in the tail
    for c in range(4):
        at(7, c, out_piece(12 + c, 0))
        at(8, c, out_piece(12 + c, 1))
        at(15, c, out_piece(12 + c, 2))
    # groups 0-2 complete at units 9, 11, 13; whole-tile transposes + proj
    for g in range(3):
        for c in range(4):
            at(10 + 2 * g, c, out_transpose(4 * g + c))
        for c in range(3):
            at(10 + 2 * g, 4 + c, out_proj(4 * g + c))
        at(11 + 2 * g, 4, out_proj(4 * g + 3))

    # ---- main pipeline, software-pipelined two slots deep ----
    exp_t = mybir.ActivationFunctionType.Exp

    slots = [(ui, jp) for ui in range(len(units)) for jp in range(JP)]
    ps_tiles = {}

    def emit_S(k):
        ui, jp = slots[k]
        ib, h = units[ui]
        rows = slice((h % 2) * 64, (h % 2) * 64 + 64)
        ps = psp.tile([128, 1024], F32, tag="ps", name="ps")
        for half in range(2):
            jt = jp * 2 + half
            nc.tensor.matmul(
                ps[:, half * 512:(half + 1) * 512],
                qkT[2 + h // 2][rows, jt * 128:(jt + 1) * 128],
                qkT[h // 2][rows, ib * 512:(ib + 1) * 512],
                start=True,
                stop=True,
            )
        ps_tiles[k] = ps

    qk_chunk(2, 0)()
    qk_chunk(0, 0)()
    emit_S(0)
    emit_S(1)
    qk_chunk(2, 1)()                          # k h0/1 chunk 1, before S(0,2)
                                              # which is emitted at slot (0,0)

    pu = None
    for k, (ui, jp) in enumerate(slots):
        ib, h = units[ui]
        ps = ps_tiles.pop(k)
        pt = ptp.tile([128, 1024], BF16, tag="pt", name="pt")
        nc.scalar.activation(pt[:], ps[:], exp_t, scale=SCALE)
        if k + 2 < len(slots):
            emit_S(k + 2)
        for f in sched.get((ui, jp), ()):
            f()
        if jp == 0:
            pu = pup.tile([128, 260], F32, tag="pu", name="pu")
        for half in range(2):
            jt = jp * 2 + half
            for c in range(4):
                # start=True clears the whole PSUM bank, so only the very
                # first matmul of the unit sets it; the other i-chunks'
                # first writes rely on per-element has_written overwrite.
                nc.tensor.matmul(
                    pu[:, c * 65:(c + 1) * 65],
                    pt[:, half * 512 + c * 128:half * 512 + (c + 1) * 128],
                    vsb[jt][:, h * 65:(h + 1) * 65],
                    start=(jt == 0 and c == 0),
                    stop=(jt == NT - 1),
                    skip_group_check=True,
                )
        if jp == JP - 1:
            for c in range(4):
                rec = work.tile([128, 1], F32, tag="rec", name="rec")
                nc.vector.reciprocal(rec[:], pu[:, c * 65 + 64:c * 65 + 65])
                nc.vector.tensor_scalar_mul(
                    Asb[ib * 4 + c][:, h * 64:(h + 1) * 64],
                    pu[:, c * 65:c * 65 + 64],
                    rec[:],
                )
    # tail: interleave the h3 transpose pieces with the projections so the
    # PE works while the DVE casts drain
    out_piece(12, 3)()
    out_piece(13, 3)()
    out_proj(12)()
    out_piece(14, 3)()
    out_proj(13)()
    out_piece(15, 3)()
    out_proj(14)()
    out_proj(15)()


@functools.lru_cache(maxsize=1)
def _build():
    nc = bacc.Bacc("TRN2", target_bir_lowering=False, debug=False,
                   num_devices=N_CORES)
    xT_d = nc.dram_tensor("xT", [C, N], BF16, kind="ExternalInput").ap()
    wqkT_d = nc.dram_tensor("wqkT", [C, 512], BF16, kind="ExternalInput").ap()
    wvT_d = nc.dram_tensor("wvT", [C, 256], BF16, kind="ExternalInput").ap()
    woT_d = nc.dram_tensor("woT", [256, C], BF16, kind="ExternalInput").ap()
    ident_d = nc.dram_tensor("ident", [128, 128], BF16, kind="ExternalInput").ap()
    out_d = nc.dram_tensor("out", [N, C], F32, kind="ExternalOutput").ap()
    from contextlib import ExitStack
    with tile.TileContext(nc) as tc, ExitStack() as ctx:
        _build_body(nc, tc, ctx, xT_d, wqkT_d, wvT_d, woT_d, ident_d, out_d)
    nc.compile()
    return nc


def _shard_inputs(x, W_qkv, W_out):
    bf16 = ml_dtypes.bfloat16
    ident = np.eye(128, dtype=bf16)
    in_maps = []
    for core in range(N_CORES):
        b, hg = core // 2, core % 2
        xT = np.ascontiguousarray(x[b].T).astype(bf16)
        rows_q = W_qkv[hg * 256:(hg + 1) * 256, :]
        rows_k = W_qkv[512 + hg * 256:512 + (hg + 1) * 256, :]
        wqkT = np.ascontiguousarray(
            np.concatenate([rows_q, rows_k], 0).T).astype(bf16)
        wvT = np.ascontiguousarray(
            W_qkv[1024 + hg * 256:1024 + (hg + 1) * 256, :].T).astype(bf16)
        woT = np.ascontiguousarray(
            W_out[:, hg * 256:(hg + 1) * 256].T).astype(bf16)
        in_maps.append(
            {"xT": xT, "wqkT": wqkT, "wvT": wvT, "woT": woT, "ident": ident})
    return in_maps


def _run(inputs, trace=False, tmpdir=None):
    x = np.asarray(inputs["x"], dtype=np.float32)
    W_qkv = np.asarray(inputs["W_qkv"], dtype=np.float32)
    W_out = np.asarray(inputs["W_out"], dtype=np.float32)
    nc = _build()
    in_maps = _shard_inputs(x, W_qkv, W_out)
    kwargs = {}
    if trace:
        kwargs = dict(trace=True, tmpdir=tmpdir)
    res = run_bass_kernel_spmd(nc, in_maps, core_ids=list(range(N_CORES)), **kwargs)
    out = np.zeros((B, N, C), np.float32)
    for core in range(N_CORES):
        out[core // 2] += res.results[core]["out"]
    return out, res


def kernel(**inputs):
    out, _ = _run(inputs)
    return out
